# revision 18
# baseline (speedup 1.0000x reference)
"""Trainium2 Bass kernel for nn_CovarianceEstimator.

Computes, for y [B=16, R=1, A=16, T=14, S=1024] complex (given as separate
real/imag f32 tensors):
  - gather P=1024 pilot positions (sym_p, sc_p) from estimation_indices
  - per-position A x A outer products sig_p sig_p^H
  - unsorted-segment-mean over subcarrier ids sc_p
  - nearest-neighbor expand via closest_subcarrier to all S subcarriers
  - broadcast over T symbols
Output: [B, R, T, S, A, A] complex64.

Sharding: data-parallel over batch; 2 batches per core on 8 cores.

The output tensor is ~470MB but holds only ~17MB of unique data: the T axis
is a pure broadcast and (for the pilot-pattern fast path) subcarrier pairs
share values and the A x A covariance is Hermitian.  The device computes and
writes only the unique data; the host does the (free) broadcast expansion,
NN pair duplication, and Hermitian mirror.

Fast path device program (pilot-pattern indices):
  - positions p = (h, s') for 2 pilot symbols x 512 even subcarriers,
    chunked [k=8][q=128] onto partitions.
  - DVE: 4 tensor_tensor muls per batch compute the Hermitian BAND
    cov[i, (i+d) % 16] for d in 0..8 (144 of 256 entries) using an
    overlapping circulant access pattern on a padded sig tile.
  - PE: identity-stationary matmuls accumulate the h-sum and the
    (aa+bb) / (ba-ab) re/im combinations directly in PSUM.
  - ACT: PSUM -> SBUF evacuation; one 576KB DMA out per batch.

Generic path (any indices): host folds segment-mean + NN-gather into a
dense [P, S] weight matrix applied on the PE (as before), but the device
writes a single [S, AA2] image per batch; host broadcasts over T.
"""

import numpy as np

B, R, A, T, S = 16, 1, 16, 14, 1024
S2 = S // 2           # even (estimated) subcarriers
P_EST = 1024          # number of (sym, sc) estimation positions
N_CORES = 8
B_LOC = B // N_CORES  # 2 batches per core
AA2 = A * A * 2       # interleaved (re, im) row payload per subcarrier
NK = 8                # position chunks of 128 (2 syms x 4 chunks of s')
ND = 9                # Hermitian band width: d = j - i mod A, d in 0..8
APAD = 32             # padded antenna axis (16 data + 9 circular + pad)
NV = A * ND           # 144 band entries per position

_cache = {}


def _fast_path_info(est, closest):
    """Return (sym0, sym1) if indices match the pilot-pattern structure:
    est == meshgrid([sym0, sym1], arange(0, S, 2)) row-major and
    closest == 2*(arange(S)//2).  Else None."""
    if est.shape != (P_EST, 2) or closest.shape != (S,):
        return None
    sc = np.arange(0, S, 2, dtype=est.dtype)
    if not np.array_equal(est[: S // 2, 1], sc):
        return None
    if not np.array_equal(est[S // 2 :, 1], sc):
        return None
    sym0 = int(est[0, 0])
    sym1 = int(est[S // 2, 0])
    if not (0 <= sym0 < T and 0 <= sym1 < T):
        return None
    if not np.all(est[: S // 2, 0] == sym0):
        return None
    if not np.all(est[S // 2 :, 0] == sym1):
        return None
    if not np.array_equal(closest, (2 * (np.arange(S) // 2)).astype(closest.dtype)):
        return None
    return sym0, sym1


def _build_fast():
    """Circulant-band fast path.  Inputs (host-prepared fp16, scaled by
    sqrt(1/2)); position p = k*128 + q -> (h, s') = (k//4, (k%4)*128 + q):
      spr, spi: [B_LOC, 128, NK, APAD]  sig re/im, circularly padded over
                antennas (24 used).
      ssr, ssi: same, shifted by one antenna (ss[i] = sp[i+1]) so odd-d
                band reads stay 4B-aligned (DVE 2x perf mode).
      nsr:      [B_LOC, 128, NK, A]  negated re (for the -a_i*b_j term).
      ident:    [128, 128] fp16 identity (PE stationary).
    Output:
      out: [B_LOC, 4, 128, 2*NV] f32 -- per s'-chunk c, partition q
           (s' = c*128+q): [re band | im band], band = (d, i) d-major.
    """
    import concourse.bacc as bacc
    import concourse.mybir as mybir
    from concourse.tile import TileContext

    f32 = mybir.dt.float32
    f16 = mybir.dt.float16
    nc = bacc.Bacc(trn_type="TRN2", target_bir_lowering=False)
    # Gauss 3-mult inputs (see _make_in_maps): per batch, piece A carries
    # k1's operands so DVE can start on a small early DMA; piece B the rest.
    #   inA: [spr | ssr | hs]           (2*NK*APAD + NK*A fp16)
    #   inB: [bs | sbs | bdn | sbdn | spi_h | na]
    LA = 2 * NK * APAD + NK * A
    LB = 4 * NK * APAD + 2 * NK * A
    inA_d = nc.declare_dram_parameter("inA", [B_LOC, 128, LA], f16, isOutput=False)
    inB_d = nc.declare_dram_parameter("inB", [B_LOC, 128, LB], f16, isOutput=False)
    id_d = nc.declare_dram_parameter("ident", [128, 128], f16, isOutput=False)
    out = nc.declare_dram_parameter("out", [B_LOC, 4, 128, 2 * NV], f16, isOutput=True)

    NDE = (ND + 1) // 2  # even d values: 0,2,4,6,8
    NDO = ND // 2        # odd d values: 1,3,5,7
    KA = NK * APAD

    with TileContext(nc) as tc:
        with (
            tc.tile_pool(name="const", bufs=1) as cp,
            tc.tile_pool(name="inp", bufs=1) as ip,
            tc.tile_pool(name="g", bufs=2) as gp,
            tc.tile_pool(name="ps", bufs=2, space="PSUM") as pp,
            tc.tile_pool(name="ev", bufs=2) as ep,
        ):
            ident = cp.tile([128, 128], f16, name="ident")
            tA = [ip.tile([128, LA], f16, name=f"tA{b}") for b in range(B_LOC)]
            tB = [ip.tile([128, LB], f16, name=f"tB{b}") for b in range(B_LOC)]
            # one queue per piece; the TT1-gating piece (inA0) on sync first
            nc.sync.dma_start(out=tA[0][:], in_=inA_d[0])
            nc.scalar.dma_start(out=tB[0][:], in_=inB_d[0])
            nc.gpsimd.dma_start(out=tA[1][:], in_=inA_d[1])
            nc.scalar.dma_start(out=ident[:], in_=id_d[:])
            nc.gpsimd.dma_start(out=tB[1][:], in_=inB_d[1])

            def seg(t, off, width):  # [128, NK, width] view
                return t[:, off : off + NK * width].rearrange(
                    "q (k a) -> q k a", k=NK, a=width
                )

            # band operand x[q, k, d, i] = base[q, k, i + 2*dh (+1 if odd)]
            def band(base, nd):  # overlapping circulant window, stride-2 d
                return type(base)(
                    base.tensor,
                    base.offset,
                    [list(p) for p in base.ap[:2]] + [[2, nd], [1, A]],
                )

            def head(x, nd):  # head_i broadcast over d (outer dim)
                return x[:, :, 0:A].unsqueeze(2).to_broadcast([128, NK, nd, A])

            for b in range(B_LOC):
                spr = seg(tA[b], 0, APAD)
                ssr = seg(tA[b], KA, APAD)
                hs = seg(tA[b], 2 * KA, A)
                bs = seg(tB[b], 0, APAD)
                sbs = seg(tB[b], KA, APAD)
                bdn = seg(tB[b], 2 * KA, APAD)
                sbdn = seg(tB[b], 3 * KA, APAD)
                sih = seg(tB[b], 4 * KA, A)
                na = seg(tB[b], 4 * KA + NK * A, A)

                # Gauss: k1 = (a+b)_i c_j;  -k3 = b_i (d-c)_j;  k2 = -a_i (c+d)_j
                # re = k1 + (-k3),  im = k1 + k2
                ga = gp.tile([128, NK, ND, A], f16, tag="ga")       # k1
                gb = gp.tile([128, NK, 2, ND, A], f16, tag="gb")    # [-k3 | k2]
                nc.vector.tensor_mul(ga[:, :, 0:ND:2], head(hs, NDE), band(spr, NDE))
                nc.vector.tensor_mul(ga[:, :, 1:ND:2], head(hs, NDO), band(ssr, NDO))
                nc.vector.tensor_mul(gb[:, :, 0, 0:ND:2], head(sih, NDE), band(bdn, NDE))
                nc.vector.tensor_mul(gb[:, :, 0, 1:ND:2], head(sih, NDO), band(sbdn, NDO))
                nc.vector.tensor_mul(gb[:, :, 1, 0:ND:2], head(na, NDE), band(bs, NDE))
                nc.vector.tensor_mul(gb[:, :, 1, 1:ND:2], head(na, NDO), band(sbs, NDO))

                ev = ep.tile([128, 4, 2 * NV], f16, tag="ev")
                pss = [pp.tile([128, 2 * NV], f32, name=f"psb{b}c{c}", tag=f"ps{c}") for c in range(4)]
                # k1 (ga) matmuls first across all chunks: they depend only
                # on the first two TTs, so the PE runs them while the gb TTs
                # are still in flight; only the 8 gb matmuls remain after
                # the last TT.
                for c in range(4):
                    for h in range(2):
                        nc.tensor.matmul(
                            pss[c][:],
                            lhsT=ident[:],
                            rhs=ga[:, 4 * h + c].rearrange("q d i -> q (d i)")
                            .unsqueeze(1).to_broadcast([128, 2, NV]),
                            start=(h == 0),
                            stop=False,
                        )
                for c in range(4):
                    for h in range(2):
                        nc.tensor.matmul(
                            pss[c][:],
                            lhsT=ident[:],
                            rhs=gb[:, 4 * h + c].rearrange("q r d i -> q (r d i)"),
                            start=False,
                            stop=(h == 1),
                        )
                    nc.scalar.copy(ev[:, c], pss[c][:])
                    if c % 2 == 1:  # ship each completed half immediately
                        nc.sync.dma_start(
                            out=out[b, c - 1 : c + 1].rearrange("c q v -> q c v"),
                            in_=ev[:, c - 1 : c + 1],
                        )
    nc.finalize()
    return nc


def _build_generic():
    """Generic program: host-gathered sig^T comes in as an input; the whole
    segment-mean + NN-gather is one dense weight matmul on the PE.
      cov[s, (i,j)] = sum_p wt[p, s] * G[p, (i,j)],  G from sig outer products.
    Device writes one [S, AA2] image per batch; host broadcasts over T.
    """
    import concourse.bacc as bacc
    import concourse.mybir as mybir
    from concourse.tile import TileContext

    f32 = mybir.dt.float32
    nc = bacc.Bacc(trn_type="TRN2", target_bir_lowering=False)
    sgr = nc.declare_dram_parameter("sgr", [B_LOC, P_EST // 128, 128, A], f32, isOutput=False)
    sgi = nc.declare_dram_parameter("sgi", [B_LOC, P_EST // 128, 128, A], f32, isOutput=False)
    wt = nc.declare_dram_parameter("wt", [P_EST, S], f32, isOutput=False)
    out = nc.declare_dram_parameter("out", [B_LOC, S, AA2], f32, isOutput=True)

    KP = P_EST // 128  # contraction chunks
    MS = S // 128      # output subcarrier chunks

    with TileContext(nc) as tc:
        with (
            tc.tile_pool(name="w", bufs=1) as wp,
            tc.tile_pool(name="sig", bufs=2) as sigp,
            tc.tile_pool(name="g", bufs=4) as gp,
            tc.tile_pool(name="ps", bufs=8, space="PSUM") as psp,
            tc.tile_pool(name="f", bufs=2) as fp,
        ):
            w_all = wp.tile([128, KP, S], f32, name="w_all")
            nc.sync.dma_start(
                out=w_all[:], in_=wt[:].rearrange("(k q) s -> q k s", k=KP, q=128)
            )
            for b in range(B_LOC):
                sr = sigp.tile([128, KP, A], f32, tag="sr")
                si = sigp.tile([128, KP, A], f32, tag="si")
                nc.sync.dma_start(out=sr[:], in_=sgr[b].rearrange("k q a -> q k a"))
                nc.sync.dma_start(out=si[:], in_=sgi[b].rearrange("k q a -> q k a"))

                f = fp.tile([128, MS, A * A, 2], f32, tag="f")
                gtiles = {}
                for k in range(KP):
                    def ii(x):
                        return x[:, k, :, None].to_broadcast([128, A, A])

                    def jj(x):
                        return x[:, k, None, :].to_broadcast([128, A, A])

                    gr = gp.tile([128, A, A], f32, tag=f"gr{k}")
                    gi = gp.tile([128, A, A], f32, tag=f"gi{k}")
                    tt = gp.tile([128, A, A], f32, tag="tt")
                    nc.vector.tensor_mul(gr[:], ii(sr), jj(sr))
                    nc.vector.tensor_mul(tt[:], ii(si), jj(si))
                    nc.vector.tensor_add(gr[:], gr[:], tt[:])
                    nc.vector.tensor_mul(gi[:], ii(si), jj(sr))
                    nc.vector.tensor_mul(tt[:], ii(sr), jj(si))
                    nc.vector.tensor_sub(gi[:], gi[:], tt[:])
                    gtiles[k] = (gr, gi)

                for m in range(MS):
                    for part in range(2):
                        ppp = psp.tile([128, A * A], f32, tag="pp")
                        for k in range(KP):
                            g = gtiles[k][part]
                            nc.tensor.matmul(
                                ppp[:],
                                lhsT=w_all[:, k, m * 128 : (m + 1) * 128],
                                rhs=g[:].rearrange("q i j -> q (i j)"),
                                start=(k == 0),
                                stop=(k == KP - 1),
                            )
                        nc.vector.tensor_copy(f[:, m, :, part], ppp[:])

                dst = out[b].rearrange(
                    "(m q) (ij ri) -> q m ij ri", m=MS, q=128, ij=A * A, ri=2
                )
                nc.sync.dma_start(out=dst, in_=f[:])
    nc.finalize()
    return nc


def _get_program(est, closest):
    key = (est.tobytes(), closest.tobytes())
    hit = _cache.get(key)
    if hit is not None:
        return hit
    fast = _fast_path_info(est, closest)
    if fast is not None:
        prog = ("fast", _build_fast(), fast)
    else:
        counts = np.zeros(S, dtype=np.float64)
        np.add.at(counts, est[:, 1], 1.0)
        denom = np.maximum(counts, 1.0)
        # wt[p, s] = [sc_p == closest[s]] / denom[closest[s]]
        wtm = (
            (est[:, 1][:, None] == closest[None, :]).astype(np.float32)
            / denom[closest][None, :].astype(np.float32)
        )
        prog = ("generic", _build_generic(), np.ascontiguousarray(wtm))
    _cache[key] = prog
    return prog


def _make_in_maps(kind, extra, yr, yi, est):
    """Build the per-core input maps for the given program kind.
    yr, yi: [B, A, T, S] f32 (R squeezed)."""
    if kind == "fast":
        sym0, sym1 = extra
        scale = np.float32(np.sqrt(0.5))
        # sig[b, h, s', a] = y[b, a, sym_h, 2 s'] * sqrt(1/2)
        def pack(y):
            s = y[:, :, (sym0, sym1), ::2]            # [B, A, 2, S2]
            s = np.transpose(s, (0, 2, 3, 1)) * scale  # [B, 2, S2, A]
            # p = k*128 + q, k = h*4 + c, s' = c*128 + q
            s = s.reshape(B, 2, 4, 128, A).transpose(0, 3, 1, 2, 4)  # [B,128,2,4,A]
            s = s.reshape(B, 128, NK, A)
            sp = np.zeros((B, 128, NK, APAD), dtype=np.float16)
            sp[..., :A] = s
            sp[..., A : A + ND - 1] = s[..., : ND - 1]
            ss = np.zeros_like(sp)
            ss[..., : A + ND - 2] = sp[..., 1 : A + ND - 1]
            return sp, ss

        spr, ssr = pack(yr)
        spi, ssi = pack(yi)
        # Gauss operands: k1 = (a+b)_i c_j, -k3 = b_i (d-c)_j, k2 = -a_i (c+d)_j
        bsum, sbsum = (spr + spi), (ssr + ssi)       # c+d band (+shifted)
        bdn, sbdn = (spi - spr), (ssi - ssr)         # d-c band (+shifted)
        hsum = spr[..., :A] + spi[..., :A]           # (a+b) head
        sih = spi[..., :A]                           # b head
        na = -spr[..., :A]                           # -a head
        KAF = NK * APAD

        def flat(x):
            return x.reshape(B, 128, -1)

        inA = np.concatenate([flat(spr), flat(ssr), flat(hsum)], axis=2)
        inB = np.concatenate(
            [flat(bsum), flat(sbsum), flat(bdn), flat(sbdn), flat(sih), flat(na)],
            axis=2,
        )
        inA = np.ascontiguousarray(inA, dtype=np.float16)
        inB = np.ascontiguousarray(inB, dtype=np.float16)
        ident = np.eye(128, dtype=np.float16)
        return [
            {
                "inA": inA[c * B_LOC : (c + 1) * B_LOC],
                "inB": inB[c * B_LOC : (c + 1) * B_LOC],
                "ident": ident,
            }
            for c in range(N_CORES)
        ]
    else:
        wtm = extra
        sym = est[:, 0].astype(np.int64)
        sc = est[:, 1].astype(np.int64)
        sgr = yr[:, :, sym, sc]  # [B, A, P]
        sgi = yi[:, :, sym, sc]
        sgr = np.ascontiguousarray(
            sgr.transpose(0, 2, 1).reshape(B, P_EST // 128, 128, A)
        )
        sgi = np.ascontiguousarray(
            sgi.transpose(0, 2, 1).reshape(B, P_EST // 128, 128, A)
        )
        return [
            {
                "sgr": sgr[c * B_LOC : (c + 1) * B_LOC],
                "sgi": sgi[c * B_LOC : (c + 1) * B_LOC],
                "wt": wtm,
            }
            for c in range(N_CORES)
        ]


_DD, _II = np.meshgrid(np.arange(ND), np.arange(A), indexing="ij")
_JJ = (_II + _DD) % A


def kernel(y_real, y_imag, estimation_indices, closest_subcarrier):
    from concourse.bass_utils import run_bass_kernel_spmd

    assert y_real.shape == (B, R, A, T, S), y_real.shape
    est = np.asarray(estimation_indices)
    closest = np.asarray(closest_subcarrier)
    kind, nc, extra = _get_program(est, closest)

    yr = np.ascontiguousarray(np.asarray(y_real, dtype=np.float32)[:, 0])
    yi = np.ascontiguousarray(np.asarray(y_imag, dtype=np.float32)[:, 0])
    in_maps = _make_in_maps(kind, extra, yr, yi, est)

    res = run_bass_kernel_spmd(nc, in_maps, list(range(N_CORES)))
    parts = [res.results[c]["out"] for c in range(N_CORES)]
    full = np.concatenate(parts, axis=0)

    if kind == "fast":
        # full: [B, 4, 128, 2*NV] fp16 -> band values v[b, s', d, i]
        full = full.reshape(B, S2, 2, ND, A).astype(np.float32)
        v = (full[:, :, 0] + 1j * full[:, :, 1]).astype(np.complex64)
        cov_half = np.empty((B, S2, A, A), dtype=np.complex64)
        cov_half[:, :, _II, _JJ] = v
        cov_half[:, :, _JJ, _II] = np.conj(v)
        cov = np.repeat(cov_half, 2, axis=1)  # NN expand to all S
    else:
        # full: [B, S, AA2] interleaved (ij, ri)
        cov = full.view(np.complex64).reshape(B, S, A, A)

    out = np.broadcast_to(
        cov.reshape(B, 1, 1, S, A, A), (B, R, T, S, A, A)
    )
    return np.ascontiguousarray(out)


# revision 19
# speedup vs baseline: 1.0542x; 1.0542x over previous
"""Trainium2 Bass kernel for nn_CovarianceEstimator.

Computes, for y [B=16, R=1, A=16, T=14, S=1024] complex (given as separate
real/imag f32 tensors):
  - gather P=1024 pilot positions (sym_p, sc_p) from estimation_indices
  - per-position A x A outer products sig_p sig_p^H
  - unsorted-segment-mean over subcarrier ids sc_p
  - nearest-neighbor expand via closest_subcarrier to all S subcarriers
  - broadcast over T symbols
Output: [B, R, T, S, A, A] complex64.

Sharding: data-parallel over batch; 2 batches per core on 8 cores.

The output tensor is ~470MB but holds only ~17MB of unique data: the T axis
is a pure broadcast and (for the pilot-pattern fast path) subcarrier pairs
share values and the A x A covariance is Hermitian.  The device computes and
writes only the unique data; the host does the (free) broadcast expansion,
NN pair duplication, and Hermitian mirror.

Fast path device program (pilot-pattern indices):
  - positions p = (h, s') for 2 pilot symbols x 512 even subcarriers,
    chunked [k=8][q=128] onto partitions.
  - DVE: 4 tensor_tensor muls per batch compute the Hermitian BAND
    cov[i, (i+d) % 16] for d in 0..8 (144 of 256 entries) using an
    overlapping circulant access pattern on a padded sig tile.
  - PE: identity-stationary matmuls accumulate the h-sum and the
    (aa+bb) / (ba-ab) re/im combinations directly in PSUM.
  - ACT: PSUM -> SBUF evacuation; one 576KB DMA out per batch.

Generic path (any indices): host folds segment-mean + NN-gather into a
dense [P, S] weight matrix applied on the PE (as before), but the device
writes a single [S, AA2] image per batch; host broadcasts over T.
"""

import numpy as np

B, R, A, T, S = 16, 1, 16, 14, 1024
S2 = S // 2           # even (estimated) subcarriers
P_EST = 1024          # number of (sym, sc) estimation positions
N_CORES = 8
B_LOC = B // N_CORES  # 2 batches per core
AA2 = A * A * 2       # interleaved (re, im) row payload per subcarrier
NK = 8                # position chunks of 128 (2 syms x 4 chunks of s')
ND = 9                # Hermitian band width: d = j - i mod A, d in 0..8
APAD = 32             # padded antenna axis (16 data + 9 circular + pad)
NV = A * ND           # 144 band entries per position

_cache = {}


def _fast_path_info(est, closest):
    """Return (sym0, sym1) if indices match the pilot-pattern structure:
    est == meshgrid([sym0, sym1], arange(0, S, 2)) row-major and
    closest == 2*(arange(S)//2).  Else None."""
    if est.shape != (P_EST, 2) or closest.shape != (S,):
        return None
    sc = np.arange(0, S, 2, dtype=est.dtype)
    if not np.array_equal(est[: S // 2, 1], sc):
        return None
    if not np.array_equal(est[S // 2 :, 1], sc):
        return None
    sym0 = int(est[0, 0])
    sym1 = int(est[S // 2, 0])
    if not (0 <= sym0 < T and 0 <= sym1 < T):
        return None
    if not np.all(est[: S // 2, 0] == sym0):
        return None
    if not np.all(est[S // 2 :, 0] == sym1):
        return None
    if not np.array_equal(closest, (2 * (np.arange(S) // 2)).astype(closest.dtype)):
        return None
    return sym0, sym1


def _build_fast():
    """Circulant-band fast path.  Inputs (host-prepared fp16, scaled by
    sqrt(1/2)); position p = k*128 + q -> (h, s') = (k//4, (k%4)*128 + q):
      spr, spi: [B_LOC, 128, NK, APAD]  sig re/im, circularly padded over
                antennas (24 used).
      ssr, ssi: same, shifted by one antenna (ss[i] = sp[i+1]) so odd-d
                band reads stay 4B-aligned (DVE 2x perf mode).
      nsr:      [B_LOC, 128, NK, A]  negated re (for the -a_i*b_j term).
      ident:    [128, 128] fp16 identity (PE stationary).
    Output:
      out: [B_LOC, 4, 128, 2*NV] f32 -- per s'-chunk c, partition q
           (s' = c*128+q): [re band | im band], band = (d, i) d-major.
    """
    import concourse.bacc as bacc
    import concourse.mybir as mybir
    from concourse.tile import TileContext

    f32 = mybir.dt.float32
    f16 = mybir.dt.float16
    nc = bacc.Bacc(trn_type="TRN2", target_bir_lowering=False)
    # Gauss 3-mult inputs (see _make_in_maps): per batch, piece A carries
    # k1's operands so DVE can start on a small early DMA; piece B the rest.
    #   inA: [spr | ssr | hs]           (2*NK*APAD + NK*A fp16)
    #   inB: [bs | sbs | bdn | sbdn | spi_h | na]
    LA = 2 * NK * APAD + NK * A
    LB = 4 * NK * APAD + 2 * NK * A
    inA_d = nc.declare_dram_parameter("inA", [B_LOC, 128, LA], f16, isOutput=False)
    inB_d = nc.declare_dram_parameter("inB", [B_LOC, 128, LB], f16, isOutput=False)
    id_d = nc.declare_dram_parameter("ident", [128, 128], f16, isOutput=False)
    out = nc.declare_dram_parameter("out", [B_LOC, 4, 128, 2 * NV], f16, isOutput=True)

    NDE = (ND + 1) // 2  # even d values: 0,2,4,6,8
    NDO = ND // 2        # odd d values: 1,3,5,7
    KA = NK * APAD

    with TileContext(nc) as tc:
        with (
            tc.tile_pool(name="const", bufs=1) as cp,
            tc.tile_pool(name="inp", bufs=1) as ip,
            tc.tile_pool(name="g", bufs=2) as gp,
            tc.tile_pool(name="ps", bufs=2, space="PSUM") as pp,
            tc.tile_pool(name="ev", bufs=2) as ep,
        ):
            ident = cp.tile([128, 128], f16, name="ident")
            tA = [ip.tile([128, LA], f16, name=f"tA{b}") for b in range(B_LOC)]
            tB = [ip.tile([128, LB], f16, name=f"tB{b}") for b in range(B_LOC)]
            # scalar's HWDGE ring shows the best first-DMA completion
            # latency; it carries batch0 (ring is FIFO, so inB0 pipelines
            # right behind inA0). gpsimd carries batch1; sync the ident.
            nc.scalar.dma_start(out=tA[0][:], in_=inA_d[0])
            nc.scalar.dma_start(out=tB[0][:], in_=inB_d[0])
            nc.gpsimd.dma_start(out=tA[1][:], in_=inA_d[1])
            nc.gpsimd.dma_start(out=tB[1][:], in_=inB_d[1])
            nc.sync.dma_start(out=ident[:], in_=id_d[:])

            def seg(t, off, width):  # [128, NK, width] view
                return t[:, off : off + NK * width].rearrange(
                    "q (k a) -> q k a", k=NK, a=width
                )

            # band operand x[q, k, d, i] = base[q, k, i + 2*dh (+1 if odd)]
            def band(base, nd):  # overlapping circulant window, stride-2 d
                return type(base)(
                    base.tensor,
                    base.offset,
                    [list(p) for p in base.ap[:2]] + [[2, nd], [1, A]],
                )

            def head(x, nd):  # head_i broadcast over d (outer dim)
                return x[:, :, 0:A].unsqueeze(2).to_broadcast([128, NK, nd, A])

            for b in range(B_LOC):
                spr = seg(tA[b], 0, APAD)
                ssr = seg(tA[b], KA, APAD)
                hs = seg(tA[b], 2 * KA, A)
                bs = seg(tB[b], 0, APAD)
                sbs = seg(tB[b], KA, APAD)
                bdn = seg(tB[b], 2 * KA, APAD)
                sbdn = seg(tB[b], 3 * KA, APAD)
                sih = seg(tB[b], 4 * KA, A)
                na = seg(tB[b], 4 * KA + NK * A, A)

                # Gauss: k1 = (a+b)_i c_j;  -k3 = b_i (d-c)_j;  k2 = -a_i (c+d)_j
                # re = k1 + (-k3),  im = k1 + k2
                ga = gp.tile([128, NK, ND, A], f16, tag="ga")       # k1
                gb = gp.tile([128, NK, 2, ND, A], f16, tag="gb")    # [-k3 | k2]
                nc.vector.tensor_mul(ga[:, :, 0:ND:2], head(hs, NDE), band(spr, NDE))
                nc.vector.tensor_mul(ga[:, :, 1:ND:2], head(hs, NDO), band(ssr, NDO))
                nc.vector.tensor_mul(gb[:, :, 0, 0:ND:2], head(sih, NDE), band(bdn, NDE))
                nc.vector.tensor_mul(gb[:, :, 0, 1:ND:2], head(sih, NDO), band(sbdn, NDO))
                nc.vector.tensor_mul(gb[:, :, 1, 0:ND:2], head(na, NDE), band(bs, NDE))
                nc.vector.tensor_mul(gb[:, :, 1, 1:ND:2], head(na, NDO), band(sbs, NDO))

                ev = ep.tile([128, 4, 2 * NV], f16, tag="ev")
                pss = [pp.tile([128, 2 * NV], f32, name=f"psb{b}c{c}", tag=f"ps{c}") for c in range(4)]
                # k1 (ga) matmuls first across all chunks: they depend only
                # on the first two TTs, so the PE runs them while the gb TTs
                # are still in flight; only the 8 gb matmuls remain after
                # the last TT.
                for c in range(4):
                    for h in range(2):
                        nc.tensor.matmul(
                            pss[c][:],
                            lhsT=ident[:],
                            rhs=ga[:, 4 * h + c].rearrange("q d i -> q (d i)")
                            .unsqueeze(1).to_broadcast([128, 2, NV]),
                            start=(h == 0),
                            stop=False,
                        )
                for c in range(4):
                    for h in range(2):
                        nc.tensor.matmul(
                            pss[c][:],
                            lhsT=ident[:],
                            rhs=gb[:, 4 * h + c].rearrange("q r d i -> q (r d i)"),
                            start=False,
                            stop=(h == 1),
                        )
                    # batch1's evacs run after the TT phase: DVE is idle
                    # then, so split them between ACT and DVE
                    if b == 1 and c % 2 == 1:
                        nc.vector.tensor_copy(ev[:, c], pss[c][:])
                    else:
                        nc.scalar.copy(ev[:, c], pss[c][:])
                    if c % 2 == 1:  # ship each completed half immediately
                        nc.sync.dma_start(
                            out=out[b, c - 1 : c + 1].rearrange("c q v -> q c v"),
                            in_=ev[:, c - 1 : c + 1],
                        )
    nc.finalize()
    return nc


def _build_generic():
    """Generic program: host-gathered sig^T comes in as an input; the whole
    segment-mean + NN-gather is one dense weight matmul on the PE.
      cov[s, (i,j)] = sum_p wt[p, s] * G[p, (i,j)],  G from sig outer products.
    Device writes one [S, AA2] image per batch; host broadcasts over T.
    """
    import concourse.bacc as bacc
    import concourse.mybir as mybir
    from concourse.tile import TileContext

    f32 = mybir.dt.float32
    nc = bacc.Bacc(trn_type="TRN2", target_bir_lowering=False)
    sgr = nc.declare_dram_parameter("sgr", [B_LOC, P_EST // 128, 128, A], f32, isOutput=False)
    sgi = nc.declare_dram_parameter("sgi", [B_LOC, P_EST // 128, 128, A], f32, isOutput=False)
    wt = nc.declare_dram_parameter("wt", [P_EST, S], f32, isOutput=False)
    out = nc.declare_dram_parameter("out", [B_LOC, S, AA2], f32, isOutput=True)

    KP = P_EST // 128  # contraction chunks
    MS = S // 128      # output subcarrier chunks

    with TileContext(nc) as tc:
        with (
            tc.tile_pool(name="w", bufs=1) as wp,
            tc.tile_pool(name="sig", bufs=2) as sigp,
            tc.tile_pool(name="g", bufs=4) as gp,
            tc.tile_pool(name="ps", bufs=8, space="PSUM") as psp,
            tc.tile_pool(name="f", bufs=2) as fp,
        ):
            w_all = wp.tile([128, KP, S], f32, name="w_all")
            nc.sync.dma_start(
                out=w_all[:], in_=wt[:].rearrange("(k q) s -> q k s", k=KP, q=128)
            )
            for b in range(B_LOC):
                sr = sigp.tile([128, KP, A], f32, tag="sr")
                si = sigp.tile([128, KP, A], f32, tag="si")
                nc.sync.dma_start(out=sr[:], in_=sgr[b].rearrange("k q a -> q k a"))
                nc.sync.dma_start(out=si[:], in_=sgi[b].rearrange("k q a -> q k a"))

                f = fp.tile([128, MS, A * A, 2], f32, tag="f")
                gtiles = {}
                for k in range(KP):
                    def ii(x):
                        return x[:, k, :, None].to_broadcast([128, A, A])

                    def jj(x):
                        return x[:, k, None, :].to_broadcast([128, A, A])

                    gr = gp.tile([128, A, A], f32, tag=f"gr{k}")
                    gi = gp.tile([128, A, A], f32, tag=f"gi{k}")
                    tt = gp.tile([128, A, A], f32, tag="tt")
                    nc.vector.tensor_mul(gr[:], ii(sr), jj(sr))
                    nc.vector.tensor_mul(tt[:], ii(si), jj(si))
                    nc.vector.tensor_add(gr[:], gr[:], tt[:])
                    nc.vector.tensor_mul(gi[:], ii(si), jj(sr))
                    nc.vector.tensor_mul(tt[:], ii(sr), jj(si))
                    nc.vector.tensor_sub(gi[:], gi[:], tt[:])
                    gtiles[k] = (gr, gi)

                for m in range(MS):
                    for part in range(2):
                        ppp = psp.tile([128, A * A], f32, tag="pp")
                        for k in range(KP):
                            g = gtiles[k][part]
                            nc.tensor.matmul(
                                ppp[:],
                                lhsT=w_all[:, k, m * 128 : (m + 1) * 128],
                                rhs=g[:].rearrange("q i j -> q (i j)"),
                                start=(k == 0),
                                stop=(k == KP - 1),
                            )
                        nc.vector.tensor_copy(f[:, m, :, part], ppp[:])

                dst = out[b].rearrange(
                    "(m q) (ij ri) -> q m ij ri", m=MS, q=128, ij=A * A, ri=2
                )
                nc.sync.dma_start(out=dst, in_=f[:])
    nc.finalize()
    return nc


def _get_program(est, closest):
    key = (est.tobytes(), closest.tobytes())
    hit = _cache.get(key)
    if hit is not None:
        return hit
    fast = _fast_path_info(est, closest)
    if fast is not None:
        prog = ("fast", _build_fast(), fast)
    else:
        counts = np.zeros(S, dtype=np.float64)
        np.add.at(counts, est[:, 1], 1.0)
        denom = np.maximum(counts, 1.0)
        # wt[p, s] = [sc_p == closest[s]] / denom[closest[s]]
        wtm = (
            (est[:, 1][:, None] == closest[None, :]).astype(np.float32)
            / denom[closest][None, :].astype(np.float32)
        )
        prog = ("generic", _build_generic(), np.ascontiguousarray(wtm))
    _cache[key] = prog
    return prog


def _make_in_maps(kind, extra, yr, yi, est):
    """Build the per-core input maps for the given program kind.
    yr, yi: [B, A, T, S] f32 (R squeezed)."""
    if kind == "fast":
        sym0, sym1 = extra
        scale = np.float32(np.sqrt(0.5))
        # sig[b, h, s', a] = y[b, a, sym_h, 2 s'] * sqrt(1/2)
        def pack(y):
            s = y[:, :, (sym0, sym1), ::2]            # [B, A, 2, S2]
            s = np.transpose(s, (0, 2, 3, 1)) * scale  # [B, 2, S2, A]
            # p = k*128 + q, k = h*4 + c, s' = c*128 + q
            s = s.reshape(B, 2, 4, 128, A).transpose(0, 3, 1, 2, 4)  # [B,128,2,4,A]
            s = s.reshape(B, 128, NK, A)
            sp = np.zeros((B, 128, NK, APAD), dtype=np.float16)
            sp[..., :A] = s
            sp[..., A : A + ND - 1] = s[..., : ND - 1]
            ss = np.zeros_like(sp)
            ss[..., : A + ND - 2] = sp[..., 1 : A + ND - 1]
            return sp, ss

        spr, ssr = pack(yr)
        spi, ssi = pack(yi)
        # Gauss operands: k1 = (a+b)_i c_j, -k3 = b_i (d-c)_j, k2 = -a_i (c+d)_j
        bsum, sbsum = (spr + spi), (ssr + ssi)       # c+d band (+shifted)
        bdn, sbdn = (spi - spr), (ssi - ssr)         # d-c band (+shifted)
        hsum = spr[..., :A] + spi[..., :A]           # (a+b) head
        sih = spi[..., :A]                           # b head
        na = -spr[..., :A]                           # -a head
        KAF = NK * APAD

        def flat(x):
            return x.reshape(B, 128, -1)

        inA = np.concatenate([flat(spr), flat(ssr), flat(hsum)], axis=2)
        inB = np.concatenate(
            [flat(bsum), flat(sbsum), flat(bdn), flat(sbdn), flat(sih), flat(na)],
            axis=2,
        )
        inA = np.ascontiguousarray(inA, dtype=np.float16)
        inB = np.ascontiguousarray(inB, dtype=np.float16)
        ident = np.eye(128, dtype=np.float16)
        return [
            {
                "inA": inA[c * B_LOC : (c + 1) * B_LOC],
                "inB": inB[c * B_LOC : (c + 1) * B_LOC],
                "ident": ident,
            }
            for c in range(N_CORES)
        ]
    else:
        wtm = extra
        sym = est[:, 0].astype(np.int64)
        sc = est[:, 1].astype(np.int64)
        sgr = yr[:, :, sym, sc]  # [B, A, P]
        sgi = yi[:, :, sym, sc]
        sgr = np.ascontiguousarray(
            sgr.transpose(0, 2, 1).reshape(B, P_EST // 128, 128, A)
        )
        sgi = np.ascontiguousarray(
            sgi.transpose(0, 2, 1).reshape(B, P_EST // 128, 128, A)
        )
        return [
            {
                "sgr": sgr[c * B_LOC : (c + 1) * B_LOC],
                "sgi": sgi[c * B_LOC : (c + 1) * B_LOC],
                "wt": wtm,
            }
            for c in range(N_CORES)
        ]


_DD, _II = np.meshgrid(np.arange(ND), np.arange(A), indexing="ij")
_JJ = (_II + _DD) % A


def kernel(y_real, y_imag, estimation_indices, closest_subcarrier):
    from concourse.bass_utils import run_bass_kernel_spmd

    assert y_real.shape == (B, R, A, T, S), y_real.shape
    est = np.asarray(estimation_indices)
    closest = np.asarray(closest_subcarrier)
    kind, nc, extra = _get_program(est, closest)

    yr = np.ascontiguousarray(np.asarray(y_real, dtype=np.float32)[:, 0])
    yi = np.ascontiguousarray(np.asarray(y_imag, dtype=np.float32)[:, 0])
    in_maps = _make_in_maps(kind, extra, yr, yi, est)

    res = run_bass_kernel_spmd(nc, in_maps, list(range(N_CORES)))
    parts = [res.results[c]["out"] for c in range(N_CORES)]
    full = np.concatenate(parts, axis=0)

    if kind == "fast":
        # full: [B, 4, 128, 2*NV] fp16 -> band values v[b, s', d, i]
        full = full.reshape(B, S2, 2, ND, A).astype(np.float32)
        v = (full[:, :, 0] + 1j * full[:, :, 1]).astype(np.complex64)
        cov_half = np.empty((B, S2, A, A), dtype=np.complex64)
        cov_half[:, :, _II, _JJ] = v
        cov_half[:, :, _JJ, _II] = np.conj(v)
        cov = np.repeat(cov_half, 2, axis=1)  # NN expand to all S
    else:
        # full: [B, S, AA2] interleaved (ij, ri)
        cov = full.view(np.complex64).reshape(B, S, A, A)

    out = np.broadcast_to(
        cov.reshape(B, 1, 1, S, A, A), (B, R, T, S, A, A)
    )
    return np.ascontiguousarray(out)
